# revision 12
# baseline (speedup 1.0000x reference)
"""Multi-head attention Bass/Tile kernel for Trainium2, 8-core SPMD.

Problem: B=4, Q=K=2048, D=512, H=8 heads (head dim 64), fp32.
  head_q = q @ Wq.T ; head_k = k @ Wk.T ; head_v = v @ Wv.T
  S = (head_q . head_k) / 8 ; masked softmax over keys ; out = (P . head_v) @ Wo.T

Sharding: data-parallel over (batch, query-half): core c handles batch c//2,
query rows (c%2)*1024 .. +1024.  Each core computes a disjoint output slice;
no collectives.

v2 design (software-pipelined single stream):
  - Masked kv rows dropped host-side; KLE=1152 static bound (count for the
    fixed seed is <=1044; 5.7 sigma for a resampled mask) with a lazy
    fallback build at KLE=2048 if a mask ever exceeds it.
  - All matmuls contract over the PE partition dim; q/k/v/W transposed
    on-chip (PE transpose in f32r mode) into d-major form.
  - Attention emits, per (ic,hp,jt) step: scores(next step) -> one
    interleaved projection/transpose group -> PV(this step).  The scalar
    engine streams exp() back-to-back ([128,1024] tiles, the global ACT
    floor) while the PE fills its shadow with the projection work, staying
    dense enough to hold the HAM clock at 8/8.
  - Softmax denominator = extra mask column in the PV stationary operand
    (lands at PSUM partition 64 for free).  Normalization: DVE reciprocal
    of that row, GpSimd partition-broadcast, one DVE multiply to evacuate.
  - Output projection contracts head PAIRS in one matmul: woT tiles
    naturally stack even/odd head rows at partitions 0:64/64:128, and A2
    tiles are written the same way.
"""

import sys

if "/opt/trn_rl_repo" not in sys.path:
    sys.path.insert(0, "/opt/trn_rl_repo")

from collections import deque
from contextlib import ExitStack

import numpy as np

import concourse.bass as bass
import concourse.tile as tile
from concourse import mybir
import bass_rust as _bass_rust

F32 = mybir.dt.float32
F32R = mybir.dt.float32r
BF16 = mybir.dt.bfloat16
EXP = mybir.ActivationFunctionType.Exp

B, Q, KL, D, H = 4, 2048, 2048, 512, 8
HD = D // H            # 64
QS = Q // 2            # 1024 query rows per core
KLE = 1152             # static padded bound on unmasked kv rows (9 tiles)
SCALE = 1.0 / HD ** 0.5
EXPBIAS = -30.0        # softmax-invariant shift; keeps exp() well-scaled


def _legalize_waits(nc, max_waits=1):
    """This walrus build only encodes one sem-wait per instruction; Tile's
    tail drain carries several.  Split extras onto preceding NoOps."""
    n = 0
    for f in nc.m.functions:
        for bb in f.blocks:
            insts = bb.instructions
            i = 0
            while i < len(insts):
                inst = insts[i]
                si = inst.sync_info
                if si is not None and len(si.on_wait) > max_waits:
                    waits = list(si.on_wait)
                    for j, w in enumerate(waits[max_waits:]):
                        nop = mybir.InstNoOp(
                            name=f"{inst.name}-waitsplit{j}", ins=[], outs=[]
                        )
                        nop.engine = inst.engine
                        nop.sync_info = _bass_rust.SyncInfo(on_wait=[w], on_update=[])
                        insts.insert(i, nop)
                        i += 1
                        n += 1
                    inst.sync_info = _bass_rust.SyncInfo(
                        on_wait=waits[:max_waits], on_update=list(si.on_update)
                    )
                i += 1
    return n


def build_kernel(kle=KLE):
    NJT = kle // 128
    nc = bass.Bass("TRN2", target_bir_lowering=False, debug=False)

    q_d = nc.dram_tensor("q", [QS, D], F32, kind="ExternalInput").ap()
    k_d = nc.dram_tensor("k", [kle, D], F32, kind="ExternalInput").ap()
    v_d = nc.dram_tensor("v", [kle, D], F32, kind="ExternalInput").ap()
    w_d = {
        w: nc.dram_tensor(w, [D, D], F32, kind="ExternalInput").ap()
        for w in ("wq", "wk", "wv", "wo")
    }
    # mask2d[p, t] = float(attn_mask[t*128 + p] != 0)
    m_d = nc.dram_tensor("mask2d", [128, NJT], F32, kind="ExternalInput").ap()
    out_d = nc.dram_tensor("out", [QS, D], F32, kind="ExternalOutput").ap()

    ident_d = nc.inline_tensor(np.eye(128, dtype=np.float32), name="ident")

    with tile.TileContext(nc) as tc, ExitStack() as ctx:
        # ---- persistent pools -------------------------------------------
        pc = ctx.enter_context(tc.tile_pool(name="const", bufs=1))
        ident = pc.tile([128, 128], F32, tag="ident")
        nc.sync.dma_start(ident[:], ident_d.ap())
        m_sb = pc.tile([128, NJT], F32, tag="m_sb")
        nc.sync.dma_start(m_sb[:], m_d)
        ebias = pc.tile([128, 1], F32, tag="ebias")
        nc.vector.memset(ebias[:], EXPBIAS)
        # warm the ACT exp table while DMAs stream in
        warm = pc.tile([128, 1], F32, tag="warm")
        nc.scalar.activation(warm[:], ebias[:], EXP)
        ones_f = pc.tile([1, HD], F32, tag="ones_f")
        nc.vector.memset(ones_f[:], 1.0)
        ones_r = pc.tile([1, HD], F32R, tag="ones_r")
        nc.vector.tensor_copy(ones_r[:], ones_f[:])

        pw = ctx.enter_context(tc.tile_pool(name="weightsT", bufs=1))
        wT = {}
        for name in ("wq", "wk", "wv"):
            wT[name] = [
                pw.tile([128, D], F32R, tag=f"{name}T{i}", name=f"{name}T{i}")
                for i in range(4)
            ]
        woT = [pw.tile([128, D], BF16, tag=f"woT{i}", name=f"woT{i}") for i in range(4)]

        pp = ctx.enter_context(tc.tile_pool(name="proj", bufs=1))
        KT = [pp.tile([128, kle], F32R, tag=f"KT{i}", name=f"KT{i}") for i in range(4)]
        QT = [pp.tile([128, QS], F32R, tag=f"QT{i}", name=f"QT{i}") for i in range(4)]
        VS = [
            pp.tile([128, H * (HD + 1)], BF16, tag=f"VS{i}", name=f"VS{i}")
            for i in range(NJT)
        ]
        A2 = [pp.tile([128, QS], BF16, tag=f"A2{i}", name=f"A2{i}") for i in range(4)]

        pact = ctx.enter_context(tc.tile_pool(name="actT", bufs=1))
        kT = [pact.tile([128, kle], F32R, tag=f"kT{i}", name=f"kT{i}") for i in range(4)]
        vT = [pact.tile([128, kle], F32R, tag=f"vT{i}", name=f"vT{i}") for i in range(4)]
        qT = [pact.tile([128, QS], F32R, tag=f"qT{i}", name=f"qT{i}") for i in range(4)]

        pst = ctx.enter_context(tc.tile_pool(name="stage", bufs=1))
        pe_pool = ctx.enter_context(tc.tile_pool(name="epool", bufs=1))
        pr = ctx.enter_context(tc.tile_pool(name="rows", bufs=1))

        # PSUM: prefix uses a scoped 4-buf pool (closed before attention);
        # attention uses s(2x2 banks) + pv(3x1) + w(1x1) = 8 banks.
        cur_ps = [None]

        def ps_w(name):
            return cur_ps[0].tile([128, 512], F32, tag="w", bufs=cur_ps[1],
                                  name=name)

        # ---- DMA: stage raw inputs (emission order = priority) ----------
        def stage_weight(name):
            tiles = []
            for ot in range(4):
                t = pst.tile([128, D], F32, tag="wstage", bufs=6,
                             name=f"wst_{name}_{ot}")
                nc.sync.dma_start(
                    t[:], w_d[name].rearrange("(t p) d -> t p d", p=128)[ot]
                )
                tiles.append(t)
            return tiles

        def stage_act(src_dram, rows, g, tag):
            """Load source 128-row tiles g*4 .. g*4+nt as [128, nt*512]."""
            nt = min(4, rows // 128 - g * 4)
            raw = pst.tile([128, 2048], F32, tag="raw", bufs=3,
                           name=f"raw_{tag}_{g}")
            nc.sync.dma_start(
                raw[:, 0:nt * 512].rearrange("p (t d) -> p t d", t=nt),
                src_dram.rearrange("(g p) d -> g p d", p=128)[g * 4:g * 4 + nt]
                .transpose([1, 0, 2]),
            )
            return raw, nt

        NKG = (NJT + 3) // 4
        wk_raw = stage_weight("wk")
        k_raw = [stage_act(k_d, kle, g, "k") for g in range(NKG)]
        wq_raw = stage_weight("wq")
        q_raw = [stage_act(q_d, QS, 0, "q")]
        wv_raw = stage_weight("wv")
        v_raw = [stage_act(v_d, kle, g, "v") for g in range(NKG)]
        q_raw.append(stage_act(q_d, QS, 1, "q"))
        wo_raw = stage_weight("wo")

        # ---- PE work generators (each yields once per emitted group) ----
        def transpose_weight(raw_tiles, dest_tiles, name):
            # dest[dt][p, ot*128+j] = W[ot*128+j, dt*128+p]
            for dt_ in range(4):
                ps = ps_w(f"psT_{name}_{dt_}")
                for ot in range(4):
                    nc.tensor.matmul(
                        ps[:, ot * 128:(ot + 1) * 128],
                        raw_tiles[ot][:, dt_ * 128:(dt_ + 1) * 128],
                        ident[:],
                        is_transpose=True,
                        start=(ot == 0),
                        stop=(ot == 3),
                    )
                nc.vector.tensor_copy(dest_tiles[dt_][:], ps[:])
                yield

        def transpose_act(raw, nt, g, xT, tag):
            # xT[dt][p, (g*4+t)*128+j] = x[(g*4+t)*128+j, dt*128+p]
            for dt_ in range(4):
                ps = ps_w(f"psT_{tag}_{g}_{dt_}")
                for t in range(nt):
                    nc.tensor.matmul(
                        ps[:, t * 128:(t + 1) * 128],
                        raw[:, t * 512 + dt_ * 128: t * 512 + dt_ * 128 + 128],
                        ident[:],
                        is_transpose=True,
                        start=(t == 0),
                        stop=(t == nt - 1),
                    )
                nc.vector.tensor_copy(
                    xT[dt_][:, g * 512:g * 512 + nt * 128], ps[:, 0:nt * 128]
                )
                yield

        def kproj(ot):
            # KT[ot][:, j] = sum_dk wkT[dk][:, ot-block].T @ kT[dk][:, j]
            # chunks of 384 (>=256 keeps f32r at 1 cy/row)
            CW = kle // 3 if kle % 3 == 0 else 512
            nch = (kle + CW - 1) // CW
            for j0 in range(0, nch, 2):
                chunks = [c for c in (j0, j0 + 1) if c < nch]
                ps = [ps_w(f"psK_{ot}_{c}") for c in chunks]
                for dk in range(4):
                    for pi, c in enumerate(chunks):
                        jc = c * CW
                        jw = min(CW, kle - jc)
                        nc.tensor.matmul(
                            ps[pi][:, 0:jw],
                            wT["wk"][dk][:, ot * 128:(ot + 1) * 128],
                            kT[dk][:, jc:jc + jw],
                            start=(dk == 0),
                            stop=(dk == 3),
                        )
                for pi, c in enumerate(chunks):
                    jc = c * CW
                    jw = min(CW, kle - jc)
                    nc.vector.tensor_copy(KT[ot][:, jc:jc + jw], ps[pi][:, 0:jw])
                yield

        def qproj(ot, ic):
            ps = ps_w(f"psQ_{ot}_{ic}")
            i0 = ic * 512
            for dk in range(4):
                nc.tensor.matmul(
                    ps[:],
                    wT["wq"][dk][:, ot * 128:(ot + 1) * 128],
                    qT[dk][:, i0:i0 + 512],
                    start=(dk == 0),
                    stop=(dk == 3),
                )
            nc.vector.tensor_copy(QT[ot][:, i0:i0 + 512], ps[:])
            yield

        def vproj(jt):
            ps = ps_w(f"psV_{jt}")
            for dk in range(4):
                nc.tensor.matmul(
                    ps[:],
                    vT[dk][:, jt * 128:(jt + 1) * 128],
                    wT["wv"][dk][:],
                    start=(dk == 0),
                    stop=(dk == 3),
                )
            vs_out = VS[jt][:].rearrange("p (h d) -> p h d", d=HD + 1)
            nc.vector.tensor_scalar(
                vs_out[:, :, 0:HD],
                ps[:].rearrange("p (h d) -> p h d", d=HD),
                m_sb[:, jt:jt + 1],
                None,
                mybir.AluOpType.mult,
            )
            nc.vector.tensor_copy(
                vs_out[:, :, HD].squeeze(),
                m_sb[:, jt:jt + 1].broadcast_to([128, H]),
            )
            yield

        def outproj(ic, it):
            # contract head pairs in one matmul: A2[hp] stacks even/odd head
            # dims at partitions 0:64/64:128, matching woT[hp] rows.
            c0 = ic * 512 + it * 128
            ps = ps_w(f"psO_{ic}_{it}")
            for hp in range(4):
                nc.tensor.matmul(
                    ps[:],
                    A2[hp][:, c0:c0 + 128],
                    woT[hp][:],
                    start=(hp == 0),
                    stop=(hp == 3),
                )
            o_sb = pr.tile([128, 512], F32, tag="osb", bufs=2, name=f"osb{ic}_{it}")
            nc.vector.tensor_copy(o_sb[:], ps[:])
            nc.sync.dma_start(out_d[c0:c0 + 128, :], o_sb[:])
            yield

        def run(gen):
            for _ in gen:
                pass

        # ---- prefix: minimum work before the first attention step -------
        with tc.tile_pool(name="psA", bufs=1, space="PSUM") as ppsA:
            cur_ps = [ppsA, 4]
            run(transpose_weight(wk_raw, wT["wk"], "wk"))
            for g, (raw, nt) in enumerate(k_raw):
                run(transpose_act(raw, nt, g, kT, "k"))
            run(kproj(0))
            run(transpose_weight(wq_raw, wT["wq"], "wq"))
            run(transpose_act(q_raw[0][0], q_raw[0][1], 0, qT, "q"))
            run(qproj(0, 0))
            run(transpose_weight(wv_raw, wT["wv"], "wv"))
            for g, (raw, nt) in enumerate(v_raw):
                run(transpose_act(raw, nt, g, vT, "v"))
            for jt in range(4):
                run(vproj(jt))

        # ---- attention with interleaved leftover projection work --------
        pps = ctx.enter_context(tc.tile_pool(name="psB", bufs=1, space="PSUM"))
        cur_ps = [pps, 1]

        def ps_s(name):
            return pps.tile([128, 1024], F32, tag="s", bufs=2, name=name)

        # interleave queue: (earliest step, generator); one group per step.
        IQ = deque()
        for jt in range(4, NJT):
            IQ.append((0, vproj(jt)))         # VS[jt] needed at step jt
        IQ.append((0, kproj(1)))              # KT[1] needed by step 9
        IQ.append((0, qproj(1, 0)))
        IQ.append((0, kproj(2)))              # step 18
        IQ.append((0, qproj(2, 0)))
        IQ.append((0, kproj(3)))              # step 27
        IQ.append((0, qproj(3, 0)))
        IQ.append((0, transpose_weight(wo_raw, woT, "wo")))
        IQ.append((0, transpose_act(q_raw[1][0], q_raw[1][1], 1, qT, "q")))
        IQ.append((0, qproj(0, 1)))           # QT ic1 by step 36
        IQ.append((0, qproj(1, 1)))
        IQ.append((0, qproj(2, 1)))
        IQ.append((0, qproj(3, 1)))
        for it in range(4):
            IQ.append((4 * NJT, outproj(0, it)))   # A2 ic0 done after step 35

        def pop_group(si):
            while IQ:
                ms, gen = IQ[0]
                if ms > si:
                    return
                try:
                    next(gen)
                    return
                except StopIteration:
                    IQ.popleft()

        steps = [(ic, hp, jt) for ic in range(2) for hp in range(4)
                 for jt in range(NJT)]

        def emit_scores(ic, hp, jt):
            i0 = ic * 512
            s = ps_s(f"s{ic}_{hp}_{jt}")
            for po2, sl in ((0, slice(0, 512)), (HD, slice(512, 1024))):
                nc.tensor.matmul(
                    s[:, sl],
                    KT[hp][po2:po2 + HD, jt * 128:(jt + 1) * 128],
                    QT[hp][po2:po2 + HD, i0:i0 + 512],
                    start=True, stop=True,
                )
            e = pe_pool.tile([128, 1024], BF16, tag="e", bufs=4,
                             name=f"e{ic}_{hp}_{jt}")
            nc.scalar.activation(e[:], s[:], EXP, scale=SCALE, bias=ebias[:, 0:1])
            return e

        def finish_pair(ic, hp, pv_e, pv_o):
            i0 = ic * 512
            d_e = pr.tile([1, 512], F32, tag="de", bufs=2, name=f"de{ic}_{hp}")
            d_o = pr.tile([1, 512], F32, tag="do", bufs=2, name=f"do{ic}_{hp}")
            nc.vector.tensor_copy(d_e[:], pv_e[64:65, :])
            nc.vector.tensor_copy(d_o[:], pv_o[64:65, :])
            nc.vector.reciprocal(d_e[:], d_e[:])
            nc.vector.reciprocal(d_o[:], d_o[:])
            r_e = pr.tile([1, 512], F32R, tag="re", bufs=2, name=f"re{ic}_{hp}")
            r_o = pr.tile([1, 512], F32R, tag="ro", bufs=2, name=f"ro{ic}_{hp}")
            nc.vector.tensor_copy(r_e[:], d_e[:])
            nc.vector.tensor_copy(r_o[:], d_o[:])
            for r_sb, pv, p0 in ((r_e, pv_e, 0), (r_o, pv_o, 64)):
                bc_ps = ps_w(f"bc{ic}_{hp}_{p0}")
                nc.tensor.matmul(
                    bc_ps[0:64, :], ones_r[0:1, 0:HD], r_sb[0:1, :],
                    start=True, stop=True,
                )
                bc_sb = pr.tile([64, 512], F32, tag="bcs", bufs=2,
                                name=f"bcs{ic}_{hp}_{p0}")
                nc.vector.tensor_copy(bc_sb[:], bc_ps[0:64, :])
                nc.vector.tensor_mul(A2[hp][p0:p0 + 64, i0:i0 + 512],
                                     pv[0:HD, :], bc_sb[:])

        e_tiles = {0: emit_scores(*steps[0])}
        pv_cur = None
        for si, (ic, hp, jt) in enumerate(steps):
            he, ho = 2 * hp, 2 * hp + 1
            # scores for the NEXT step first (keeps ACT streaming)
            if si + 1 < len(steps):
                e_tiles[si + 1] = emit_scores(*steps[si + 1])
            pop_group(si)
            if jt == 0:
                pv_cur = (
                    pps.tile([65, 512], F32, tag="pv", bufs=3, name=f"pve{ic}_{hp}"),
                    pps.tile([65, 512], F32, tag="pv", bufs=3, name=f"pvo{ic}_{hp}"),
                )
            e = e_tiles.pop(si)
            nc.tensor.matmul(
                pv_cur[0][0:65, :],
                VS[jt][:, he * (HD + 1):(he + 1) * (HD + 1)],
                e[:, 0:512],
                start=(jt == 0), stop=(jt == NJT - 1),
            )
            nc.tensor.matmul(
                pv_cur[1][0:65, :],
                VS[jt][:, ho * (HD + 1):(ho + 1) * (HD + 1)],
                e[:, 512:1024],
                start=(jt == 0), stop=(jt == NJT - 1),
            )
            if jt == NJT - 1:
                finish_pair(ic, hp, pv_cur[0], pv_cur[1])

        # drain leftover interleave work, then ic1 output projection
        while IQ:
            pop_group(10 ** 9)
        for it in range(4):
            run(outproj(1, it))

    return nc


_NC_CACHE = {}


def _get_nc(kle):
    if kle not in _NC_CACHE:
        nc = build_kernel(kle)
        _legalize_waits(nc)
        _NC_CACHE[kle] = nc
    return _NC_CACHE[kle]


def shard_inputs(query, key, value, Wq, Wk, Wv, Wo, attn_mask, kle=KLE):
    """Per-core shards.  Masked kv rows are dropped (order-invariant under
    softmax; fully-masked rows contribute exactly 0) and the rest packed
    into a static kle-row buffer, zero-padded with mask=0."""
    in_maps = []
    for c in range(8):
        b, half = c // 2, c % 2
        m = np.asarray(attn_mask[b]) != 0
        idx = np.nonzero(m)[0]
        if len(idx) > kle:
            raise ValueError(f"unmasked count {len(idx)} exceeds kle={kle}")
        kc = np.zeros((kle, D), np.float32)
        vc = np.zeros((kle, D), np.float32)
        kc[: len(idx)] = np.asarray(key[b])[idx]
        vc[: len(idx)] = np.asarray(value[b])[idx]
        mf = np.zeros(kle, np.float32)
        mf[: len(idx)] = 1.0
        in_maps.append({
            "q": np.ascontiguousarray(query[b, half * QS:(half + 1) * QS]),
            "k": kc,
            "v": vc,
            "wq": np.asarray(Wq), "wk": np.asarray(Wk),
            "wv": np.asarray(Wv), "wo": np.asarray(Wo),
            "mask2d": np.ascontiguousarray(mf.reshape(kle // 128, 128).T),
        })
    return in_maps


def kernel(query, key, value, Wq, Wk, Wv, Wo, attn_mask, _trace=False, _trace_kwargs=None):
    from concourse.bass_utils import run_bass_kernel_spmd

    query = np.asarray(query, dtype=np.float32)
    key = np.asarray(key, dtype=np.float32)
    value = np.asarray(value, dtype=np.float32)
    counts = (np.asarray(attn_mask) != 0).sum(axis=1)
    kle = KLE if counts.max() <= KLE else 2048
    in_maps = shard_inputs(query, key, value, Wq, Wk, Wv, Wo, attn_mask, kle)
    nc = _get_nc(kle)
    res = run_bass_kernel_spmd(
        nc, in_maps, list(range(8)), trace=_trace, **(_trace_kwargs or {})
    )
    out = np.empty((B, Q, D), dtype=np.float32)
    for c in range(8):
        b, half = c // 2, c % 2
        out[b, half * QS:(half + 1) * QS] = res.results[c]["out"]
    if _trace:
        kernel._last_results = res
    return out
